# revision 11
# baseline (speedup 1.0000x reference)
"""Trainium2 Bass kernel for nn_BrocaModule (MoE routing, dense-expert form).

Strategy: data-parallel over the batch across 8 NeuronCores (2048 tokens each).
On-chip layout is feature-major (features on SBUF partitions, tokens on the
free dim) so every GEMM contracts along partitions.  Per 512-token chunk:

  emb -> (PE transpose) embT -> proj+gate GEMMs -> top-2 gating math
  -> per-expert liquid GEMM + fused tanh (ACT) -> gate-mask multiply (DVE)
  -> accumulated expert-out GEMM (PSUM) -> tanh -> memory MLP -> surprise.

All large matmuls run as float32r (full-rate fp32 on the PE at free-dim 512);
every tile consumed by an fp32r matmul is declared float32r and written as
such by its producer (the BIR verifier requires rounded-on-write operands).
The gate mask is applied as  combined = sum_e W_out'[e].T @ (h_e * m_e)  with
W_out'[e] = diag(DT/(1+DT/tau[e])) @ W_out[e] (scaled once on device), and the
b_out term enters the same PSUM accumulation as an extra K=8 matmul against
the mask rows.
"""

import numpy as np
from contextlib import ExitStack

import concourse.bass as bass
import concourse.tile as tile
from concourse import bacc, mybir
from concourse.bass_utils import run_bass_kernel_spmd
from concourse.masks import make_identity

B, D, S, H, E = 16384, 256, 64, 512, 8
DT = 0.2
N_CORES = 8
BC = B // N_CORES            # tokens per core
NCH = 512                    # tokens per chunk (= fp32 matmul free-dim max)
NCHUNKS = BC // NCH
F32 = mybir.dt.float32
F32R = mybir.dt.float32r
MMDT = F32R  # matmul operand dtype: F32 (exact, 4 cyc/row) or F32R (1 cyc/row)
AF = mybir.ActivationFunctionType
OP = mybir.AluOpType

PAIRED = False
DEBUG = False               # row-tiled expert-in pairs (tile_position packing)

_cache = {}


def _f(ap):
    return ap.bitcast(F32)


def _emit(ctx, tc, I, O, repeat=1):
    nc = tc.nc

    consts = ctx.enter_context(tc.tile_pool(name="consts", bufs=1))
    wts = ctx.enter_context(tc.tile_pool(name="wts", bufs=1))
    io = ctx.enter_context(tc.tile_pool(name="io", bufs=2))
    feat = ctx.enter_context(tc.tile_pool(name="feat", bufs=2))
    gsm = ctx.enter_context(tc.tile_pool(name="gsm", bufs=2))
    hpool = ctx.enter_context(tc.tile_pool(name="hpool", bufs=6))
    hmpool = ctx.enter_context(tc.tile_pool(name="hmpool", bufs=8))
    mbcp = ctx.enter_context(tc.tile_pool(name="mbcp", bufs=4))
    # PSUM: 8 banks total.
    psc = ctx.enter_context(tc.tile_pool(name="psc", bufs=2, space="PSUM"))
    prep = ctx.enter_context(tc.tile_pool(name="prep", bufs=3, space="PSUM"))
    tpp = ctx.enter_context(tc.tile_pool(name="tpp", bufs=2, space="PSUM"))
    miscp = ctx.enter_context(tc.tile_pool(name="miscp", bufs=1, space="PSUM"))

    # ---- constants ----
    ident = consts.tile([128, 128], F32)
    make_identity(nc, ident)
    onesf = consts.tile([128, 1], F32)
    nc.gpsimd.memset(onesf, 1.0)
    ones = consts.tile([128, 1], MMDT)
    nc.vector.tensor_copy(ones[:], onesf[:])

    # ---- weights to SBUF ----
    wp = wts.tile([128, 2, 128], F32)           # W_proj dup'd: [k, kt, m]
    nc.sync.dma_start(wp[:], I["wp"].rearrange("(kt k) m -> k kt m", k=128))
    bp = wts.tile([128, 1], F32)
    nc.sync.dma_start(bp[:], I["bp"])
    wg = wts.tile([64, 8], F32)
    nc.sync.dma_start(wg[:], I["wg"])
    bg = wts.tile([8, 1], F32)
    nc.sync.dma_start(bg[:], I["bg"])
    if PAIRED:
        win = wts.tile([128, E, 2, 128], MMDT)
    else:
        win = wts.tile([64, E, 4, 128], MMDT)
    nc.sync.dma_start(win[:], I["win"])
    bh = wts.tile([128, E, 4], F32)
    nc.sync.dma_start(bh[:], I["bh"])
    tau = wts.tile([128, E, 4], F32)
    nc.sync.dma_start(tau[:], I["tau"])
    wout = wts.tile([128, E, 4, 2, 128], MMDT)
    nc.sync.dma_start(wout[:], I["wout"])
    bout = wts.tile([8, 256], MMDT)
    nc.sync.dma_start(bout[:], I["bout"])
    w1 = wts.tile([128, 2, 2, 128], MMDT)
    nc.sync.dma_start(w1[:], I["w1"])
    b1 = wts.tile([128, 2], F32)
    nc.sync.dma_start(b1[:], I["b1"])
    w2 = wts.tile([128, 2, 2, 128], MMDT)
    nc.sync.dma_start(w2[:], I["w2"])
    b2 = wts.tile([128, 2], F32)
    nc.sync.dma_start(b2[:], I["b2"])

    # s[e,h] = DT / (1 + DT/tau): scale W_out rows by s once (rounds to f32r).
    sten = wts.tile([128, E, 4], F32)
    nc.vector.reciprocal(sten[:], tau[:])                       # 1/tau
    nc.vector.tensor_scalar(sten[:], sten[:], DT, 1.0, OP.mult, OP.add)
    nc.vector.reciprocal(sten[:], sten[:])                      # 1/(1+DT/tau)
    nc.vector.tensor_scalar(sten[:], sten[:], DT, None, OP.mult)
    for e in range(E):
        for k in range(4):
            nc.vector.tensor_scalar(
                wout[:, e, k, :, :], _f(wout[:, e, k, :, :]),
                sten[:, e, k:k + 1], None, OP.mult)

    emb, comb_o, surp_o = I["emb"], O["combined"], O["surprise"]

    for c in [cc for _ in range(repeat) for cc in range(NCHUNKS)]:
        t0 = c * NCH
        # ---- load + transpose embeddings: embT[d, t] ----
        embc = io.tile([128, 4, 256], F32, tag="embc")
        nc.sync.dma_start(
            embc[:], emb[t0:t0 + NCH, :].rearrange("(tb p) d -> p tb d", p=128))
        embT = feat.tile([128, 2, 4, 128], F32, tag="embT")
        for tb in range(4):
            for db in range(2):
                pt = tpp.tile([128, 128], F32, tag="tp")
                nc.tensor.transpose(pt[:], embc[:, tb, db * 128:(db + 1) * 128],
                                    ident[:])
                nc.scalar.activation(embT[:, db, tb, :], pt[:], AF.Copy)

        # ---- proj (dup'd to 128 partitions) + gate ----
        px = miscp.tile([128, NCH], F32, tag="misc")
        nc.tensor.matmul(px[:], wp[:, 0, :], embT[:, 0, :, :],
                         start=True, stop=False)
        nc.tensor.matmul(px[:], wp[:, 1, :], embT[:, 1, :, :],
                         start=False, stop=True)
        x = feat.tile([128, NCH], F32, tag="x")
        nc.scalar.activation(x[:], px[:], AF.Identity, bias=bp[:])
        xr = feat.tile([128, NCH], MMDT, tag="xr")
        nc.vector.tensor_copy(xr[:], x[:])

        if DEBUG and c == 0:
            nc.sync.dma_start(O["dbg_embT"][:], _f(embT[:]))
            nc.sync.dma_start(O["dbg_x"][:], _f(x[:]))
        pl = miscp.tile([8, NCH], F32, tag="misc")
        nc.tensor.matmul(pl[:], wg[:], x[0:64, :], start=True, stop=True)
        lT = gsm.tile([8, NCH], F32, tag="lT")
        nc.scalar.activation(lT[:], pl[:], AF.Identity, bias=bg[:])

        # ---- gating: transpose logits to token-major, top-2 softmax mask ----
        ltok = gsm.tile([128, 4, E], F32, tag="ltok")
        for tb in range(4):
            pt = tpp.tile([128, 8], F32, tag="tp")
            nc.tensor.transpose(pt[:], lT[:, tb * 128:(tb + 1) * 128],
                                ident[0:8, 0:8])
            nc.vector.tensor_copy(ltok[:, tb, :], pt[:])

        a1 = gsm.tile([128, 4], F32, tag="a1")
        nc.vector.tensor_reduce(a1[:], ltok[:], mybir.AxisListType.X, OP.max)
        a1b = a1[:, :, None].broadcast_to([128, 4, E])
        eq = gsm.tile([128, 4, E], F32, tag="eq")
        nc.vector.tensor_tensor(eq[:], ltok[:], a1b, OP.is_equal)
        nc.vector.tensor_scalar(eq[:], eq[:], -1e30, None, OP.mult)
        l2 = gsm.tile([128, 4, E], F32, tag="l2")
        nc.vector.tensor_add(l2[:], ltok[:], eq[:])
        a2 = gsm.tile([128, 4], F32, tag="a2")
        nc.vector.tensor_reduce(a2[:], l2[:], mybir.AxisListType.X, OP.max)
        dba = gsm.tile([128, 4], F32, tag="dba")
        nc.vector.tensor_sub(dba[:], a2[:], a1[:])
        edba = gsm.tile([128, 4], F32, tag="edba")
        nc.scalar.activation(edba[:], dba[:], AF.Exp)
        nc.vector.tensor_scalar(edba[:], edba[:], 1.0, None, OP.add)
        rden = gsm.tile([128, 4], F32, tag="rden")
        nc.vector.reciprocal(rden[:], edba[:])

        du = gsm.tile([128, 4, E], F32, tag="du")
        nc.vector.tensor_tensor(du[:], ltok[:], a1b, OP.subtract)
        eu = gsm.tile([128, 4, E], F32, tag="eu")
        nc.scalar.activation(eu[:], du[:], AF.Exp)
        sel = gsm.tile([128, 4, E], F32, tag="sel")
        nc.vector.tensor_tensor(sel[:], ltok[:],
                                a2[:, :, None].broadcast_to([128, 4, E]),
                                OP.is_ge)
        mtok = gsm.tile([128, 4, E], F32, tag="mtok")
        nc.vector.tensor_mul(mtok[:], eu[:], sel[:])
        nc.vector.tensor_tensor(mtok[:], mtok[:],
                                rden[:, :, None].broadcast_to([128, 4, E]),
                                OP.mult)

        mT = gsm.tile([8, 4, 128], MMDT, tag="mT")
        for tb in range(4):
            pt = tpp.tile([8, 128], F32, tag="tp")
            nc.tensor.transpose(pt[:], mtok[:, tb, :], ident[:])
            nc.vector.tensor_copy(mT[:, tb, :], pt[:])
        # gpsimd partition_broadcast needs its source on partition 0: pack the
        # 8 mask rows into one partition via an SBUF->SBUF DMA.
        mrow = gsm.tile([1, E, 4, 128], MMDT, tag="mrow")
        nc.sync.dma_start(mrow[:], mT[:])

        if DEBUG and c == 0:
            nc.sync.dma_start(O["dbg_l"][:], lT[:])
            nc.sync.dma_start(O["dbg_m"][:], _f(mrow[:]).rearrange("p e t r -> p (e t r)"))
        # ---- expert-out accumulator: start with b_out contribution ----
        pc = [psc.tile([128, NCH], F32, tag="pc", name=f"pc{c}_{m}")
              for m in range(2)]
        for m in range(2):
            nc.tensor.matmul(pc[m][:], bout[:, m * 128:(m + 1) * 128],
                             mT[:, :, :], start=True, stop=False)

        # ---- experts ----
        for e in range(E):
            mbc = mbcp.tile([128, NCH], MMDT, tag="mbc")
            nc.gpsimd.partition_broadcast(mbc[:], mrow[:, e, :, :])

            hm = []
            for j in range(2):
                if PAIRED:
                    pA = prep.tile([128, NCH], F32, tag="pre")
                    pB = prep.tile([128, NCH], F32, tag="pre")
                    nc.tensor.matmul(pA[:], win[0:64, e, j, :], xr[0:64, :],
                                     start=True, stop=True, tile_position=(0, 0))
                    nc.tensor.matmul(pB[:], win[64:128, e, j, :], xr[64:128, :],
                                     start=True, stop=True, tile_position=(64, 0))
                    pre_tiles = [pA, pB]
                else:
                    pre_tiles = []
                    for q in range(2):
                        pq = prep.tile([128, NCH], F32, tag="pre")
                        nc.tensor.matmul(pq[:], win[:, e, 2 * j + q, :],
                                         xr[0:64, :], start=True, stop=True)
                        pre_tiles.append(pq)
                for q in range(2):
                    ht = 2 * j + q
                    hti = hpool.tile([128, NCH], F32, tag="h")
                    nc.scalar.activation(hti[:], pre_tiles[q][:], AF.Tanh,
                                         bias=bh[:, e, ht:ht + 1])
                    hmt = hmpool.tile([128, NCH], MMDT, tag="hm")
                    nc.vector.tensor_mul(hmt[:], hti[:], _f(mbc[:]))
                    hm.append(hmt)
                    if DEBUG and c == 0 and e == 0 and ht == 0:
                        nc.sync.dma_start(O["dbg_h"][:], hti[:])
                        nc.sync.dma_start(O["dbg_hm"][:], _f(hmt[:]))

            for m in range(2):
                for k in range(4):
                    nc.tensor.matmul(pc[m][:], wout[:, e, k, m, :], hm[k][:],
                                     start=False,
                                     stop=(e == E - 1 and k == 3))

        comb = feat.tile([128, 2, NCH], MMDT, tag="comb")
        for m in range(2):
            nc.scalar.activation(comb[:, m, :], pc[m][:], AF.Tanh)

        if DEBUG and c == 0:
            nc.sync.dma_start(O["dbg_comb"][:], _f(comb[:]))
        # ---- memory MLP + surprise ----
        tmlp = feat.tile([128, 2, NCH], MMDT, tag="tmlp")
        for m in range(2):
            pt1 = miscp.tile([128, NCH], F32, tag="misc")
            nc.tensor.matmul(pt1[:], w1[:, 0, m, :], comb[:, 0, :],
                             start=True, stop=False)
            nc.tensor.matmul(pt1[:], w1[:, 1, m, :], comb[:, 1, :],
                             start=False, stop=True)
            nc.scalar.activation(tmlp[:, m, :], pt1[:], AF.Tanh,
                                 bias=b1[:, m:m + 1])
        sq = feat.tile([128, 2, NCH], MMDT, tag="sq")
        for m in range(2):
            pp = miscp.tile([128, NCH], F32, tag="misc")
            nc.tensor.matmul(pp[:], w2[:, 0, m, :], tmlp[:, 0, :],
                             start=True, stop=False)
            nc.tensor.matmul(pp[:], w2[:, 1, m, :], tmlp[:, 1, :],
                             start=False, stop=True)
            df = feat.tile([128, NCH], F32, tag="df")
            nc.vector.tensor_sub(df[:], pp[:], _f(comb[:, m, :]))
            nc.vector.tensor_scalar(df[:], df[:], b2[:, m:m + 1], None, OP.add)
            nc.vector.tensor_mul(sq[:, m, :], df[:], df[:])

        if DEBUG and c == 0:
            nc.sync.dma_start(O["dbg_tmlp"][:], _f(tmlp[:]))
            nc.sync.dma_start(O["dbg_sq"][:], _f(sq[:]))
        ps = miscp.tile([1, NCH], F32, tag="misc")
        nc.tensor.matmul(ps[:], ones[:], sq[:, 0, :], start=True, stop=False)
        nc.tensor.matmul(ps[:], ones[:], sq[:, 1, :], start=False, stop=True)
        surp = gsm.tile([1, NCH], F32, tag="surp")
        nc.scalar.activation(surp[:], ps[:], AF.Copy, scale=1.0 / D)
        nc.sync.dma_start(surp_o[0:1, t0:t0 + NCH], surp[:])

        # ---- transpose combined back to token-major and store ----
        outT = feat.tile([128, 4, 2, 128], F32, tag="outT")
        for tb in range(4):
            for m in range(2):
                pt = tpp.tile([128, 128], F32, tag="tp")
                nc.tensor.transpose(pt[:], _f(comb[:, m, tb * 128:(tb + 1) * 128]),
                                    ident[:])
                nc.scalar.activation(outT[:, tb, m, :], pt[:], AF.Copy)
            nc.sync.dma_start(comb_o[t0 + tb * 128:t0 + (tb + 1) * 128, :],
                              outT[:, tb, :, :])


def _build(repeat=1):
    key = f"nc{repeat}"
    if key in _cache:
        return _cache[key]
    nc = bacc.Bacc("TRN2", target_bir_lowering=False, debug=False,
                   num_devices=N_CORES)
    f = mybir.dt.float32
    fr = MMDT
    ins = {
        "emb": nc.dram_tensor("emb", [BC, D], f, kind="ExternalInput").ap(),
        "wp": nc.dram_tensor("wp", [D, 128], f, kind="ExternalInput").ap(),
        "bp": nc.dram_tensor("bp", [128, 1], f, kind="ExternalInput").ap(),
        "wg": nc.dram_tensor("wg", [S, E], f, kind="ExternalInput").ap(),
        "bg": nc.dram_tensor("bg", [E, 1], f, kind="ExternalInput").ap(),
        "win": nc.dram_tensor(
            "win", [128, E, 2, 128] if PAIRED else [64, E, 4, 128], fr,
            kind="ExternalInput").ap(),
        "bh": nc.dram_tensor("bh", [128, E, 4], f, kind="ExternalInput").ap(),
        "tau": nc.dram_tensor("tau", [128, E, 4], f, kind="ExternalInput").ap(),
        "wout": nc.dram_tensor("wout", [128, E, 4, 2, 128], fr,
                               kind="ExternalInput").ap(),
        "bout": nc.dram_tensor("bout", [E, D], fr, kind="ExternalInput").ap(),
        "w1": nc.dram_tensor("w1", [128, 2, 2, 128], fr,
                             kind="ExternalInput").ap(),
        "b1": nc.dram_tensor("b1", [128, 2], f, kind="ExternalInput").ap(),
        "w2": nc.dram_tensor("w2", [128, 2, 2, 128], fr,
                             kind="ExternalInput").ap(),
        "b2": nc.dram_tensor("b2", [128, 2], f, kind="ExternalInput").ap(),
    }
    outs = {
        "combined": nc.dram_tensor("combined", [BC, D], f,
                                   kind="ExternalOutput").ap(),
        "surprise": nc.dram_tensor("surprise", [1, BC], f,
                                   kind="ExternalOutput").ap(),
    }
    if DEBUG:
        for nm, shp in [("dbg_embT", [128, 2, 4, 128]), ("dbg_x", [128, 512]),
                        ("dbg_l", [8, 512]), ("dbg_m", [1, 4096]),
                        ("dbg_h", [128, 512]), ("dbg_hm", [128, 512]),
                        ("dbg_comb", [128, 2, 512]), ("dbg_tmlp", [128, 2, 512]),
                        ("dbg_sq", [128, 2, 512])]:
            outs[nm] = nc.dram_tensor(nm, shp, f, kind="ExternalOutput").ap()
    with tile.TileContext(nc) as tc:
        with ExitStack() as ctx:
            _emit(ctx, tc, ins, outs, repeat=repeat)
    nc.compile()
    _cache[key] = nc
    return nc


def _prep_weights(inputs):
    f = np.float32
    W_proj = np.asarray(inputs["W_proj"], f)
    W_in = np.asarray(inputs["W_in"], f)
    W_out = np.asarray(inputs["W_out"], f)
    W1 = np.asarray(inputs["W1"], f)
    W2 = np.asarray(inputs["W2"], f)
    if PAIRED:
        win = np.ascontiguousarray(
            W_in.reshape(E, S, 2, 2, 128).transpose(3, 1, 0, 2, 4)
            .reshape(128, E, 2, 128))
    else:
        win = np.ascontiguousarray(W_in.reshape(E, S, 4, 128).transpose(1, 0, 2, 3))
    return {
        "wp": np.ascontiguousarray(np.concatenate([W_proj, W_proj], axis=1)),
        "bp": np.ascontiguousarray(
            np.tile(np.asarray(inputs["b_proj"], f), 2)[:, None]),
        "wg": np.ascontiguousarray(np.asarray(inputs["W_gate"], f)),
        "bg": np.ascontiguousarray(np.asarray(inputs["b_gate"], f)[:, None]),
        "win": win,
        "bh": np.ascontiguousarray(
            np.asarray(inputs["b_h"], f).reshape(E, 4, 128).transpose(2, 0, 1)),
        "tau": np.ascontiguousarray(
            np.asarray(inputs["tau"], f).reshape(E, 4, 128).transpose(2, 0, 1)),
        "wout": np.ascontiguousarray(
            W_out.reshape(E, 4, 128, 2, 128).transpose(2, 0, 1, 3, 4)),
        "bout": np.ascontiguousarray(np.asarray(inputs["b_out"], f)),
        "w1": np.ascontiguousarray(
            W1.reshape(2, 128, 2, 128).transpose(1, 0, 2, 3)),
        "b1": np.ascontiguousarray(np.asarray(inputs["b1"], f).reshape(2, 128).T),
        "w2": np.ascontiguousarray(
            W2.reshape(2, 128, 2, 128).transpose(1, 0, 2, 3)),
        "b2": np.ascontiguousarray(np.asarray(inputs["b2"], f).reshape(2, 128).T),
    }


last_results = None


def kernel(**inputs):
    global last_results
    nc = _build()
    wmap = _prep_weights(inputs)
    emb = np.ascontiguousarray(np.asarray(inputs["embedding"], np.float32))
    in_maps = []
    for i in range(N_CORES):
        m = dict(wmap)
        m["emb"] = emb[i * BC:(i + 1) * BC]
        in_maps.append(m)
    res = run_bass_kernel_spmd(nc, in_maps, core_ids=list(range(N_CORES)))
    last_results = res
    combined = np.concatenate([res.results[i]["combined"]
                               for i in range(N_CORES)], axis=0)
    surprise = np.concatenate([res.results[i]["surprise"].reshape(-1)
                               for i in range(N_CORES)], axis=0)
    return combined.astype(np.float32), surprise.astype(np.float32)
